# revision 53
# baseline (speedup 1.0000x reference)
"""Trainium2 Bass kernel for nn_EnhancedTextAttentionBlock (v2).

Self-contained: takes FULL inputs (as in reference.setup_inputs()), shards
across 8 NeuronCores internally, returns the FULL [2, 256, 48, 48] output.

Sharding: core c handles batch b = c // 4 and query-token block k = c % 4
(576 of the 2304 spatial tokens). K/V are computed for the full token set on
every core; a single SPMD program serves all 8 cores with no collectives.

Algebraic restructurings (exact, not approximations):
- pe depends only on (c, w): the 3x3 conv collapses to three 1-D convs (bf16).
- LayerNorms are FUSED into the projections: with kn = (tok - mu)*rs*g + b,
  kst = rs ⊙ (kwg^T tok - mu ⊗ kwg_sum) where kwg = kw*g.  The mu-correction
  rides as an extra K=1 matmul accumulation chunk; the per-token rs rides as
  the exp()'s per-partition scale (keys) or is folded into qst (queries).
  kn/qn are never materialized.
- The k-projection bias shifts every score of a query by a constant and
  cancels in softmax -> dropped.  v's LN beta and bias commute through the
  softmax-normalized attention -> folded into the output bias on host.
- Softmax denominators ride as a ones-column of v; av is normalized before
  a head-grouped output projection (4 heads packed into K=128).
- Softmax max-subtraction is skipped: |scores| < ~2 in fp32 exp range.
- Channel-wise token stats are computed as matmul COLUMNS (free-dim 1), so
  the whole stats scalar chain runs on [128, nchunk] tiles (~free).
"""
import math
import numpy as np

import concourse.bass as bass
import concourse.tile as tile
from concourse import bacc, mybir
from concourse.bass_utils import run_bass_kernel_spmd

import os as _os
F32 = mybir.dt.float32
BF16 = mybir.dt.bfloat16
R = mybir.dt.float32r
_PREC = _os.environ.get("KERNEL_PREC", "allr")
PROJ_DT = F32 if _PREC == "f32" else R      # q/k/v projections (moving = tok)
SCORES_DT = F32 if _PREC in ("f32", "sf32") else R
AV_DT = F32 if _PREC in ("f32", "af32") else BF16   # v / exp(scores) storage
OPROJ_DT = BF16 if _PREC == "obf" else F32
AF = mybir.ActivationFunctionType
OP = mybir.AluOpType

B, C, H, W, T = 2, 256, 48, 48, 512
NH, HD = 8, 32
S = H * W              # 2304 tokens
NQ = S // 4            # 576 q tokens per core
SCALE = HD ** -0.5
IT = 288               # q-tile (two per q block)
MC = 96                # epilogue chunk
MEGA = 1152            # k-side pipeline chunk (2 per S)
NCH = S // MEGA        # 2 mega chunks
NJM = MEGA // 128      # 9 key chunks per mega
EPS = 1e-5

# cvecs column indices (c-major [256, 1] vectors packed into one input)
CV_TMB1, CV_L1G, CV_L1B, CV_TMB2, CV_L2GN, CV_L2BN, CV_CONVB, CV_QBT = range(8)
# rowvecs (token-major prebroadcast [128, 256] rows)
RV_OB, RV_NOG = range(2)


def build_bass():
    nc = bacc.Bacc("TRN2", target_bir_lowering=False, debug=False,
                   enable_asserts=True, num_devices=8)
    di = {}

    def inp(name, shape, dt=F32):
        di[name] = nc.dram_tensor(name, shape, dt, kind="ExternalInput")
        return di[name]

    inp("xk", [C, S])
    inp("xq", [C, NQ])
    inp("xqres", [NQ, C])          # xq^T + no_b (host-folded)
    inp("text", [T, 1])
    inp("tmw1", [T, C])
    inp("tmw2", [C, C])
    inp("cvecs", [C, 8])
    inp("pe", [C, W])
    inp("w3b", [3, 768, C], PROJ_DT)
    inp("qwg", [C, C], PROJ_DT)    # [c, d] = q_w[d, c] * nq_g[c]
    inp("kwg", [C, C], PROJ_DT)
    inp("vwg", [C, C], PROJ_DT)    # [c, o] = v_w[o, c] * nkv_g[c]
    inp("owg", [128, 2, C], OPROJ_DT)
    inp("rowvecs", [128, 2, C])
    inp("vsn", [1, C], PROJ_DT)
    inp("sumrows", [1, 2, 2, 128], PROJ_DT)  # [0, r, dc, d]: r0 = -kwg_sum, r1 = -qwg_sum
    inp("gwg", [C, 1])
    inp("gvec", [MC, 2])           # bc cols: gwg_sum, gb_total
    inp("ident", [MC, MC])
    inp("selmask", [128, 2, NQ])
    y = nc.dram_tensor("y", [NQ, C], F32, kind="ExternalOutput")
    if _os.environ.get("KERNEL_DBG") == "1":
        di["dbg_tok"] = nc.dram_tensor("dbg_tok", [128, 2, S], F32, kind="ExternalOutput")
        di["dbg_kst"] = nc.dram_tensor("dbg_kst", [128, 2, S], SCORES_DT, kind="ExternalOutput")
        di["dbg_gate"] = nc.dram_tensor("dbg_gate", [MC, 6], F32, kind="ExternalOutput")
        di["dbg_qst"] = nc.dram_tensor("dbg_qst", [128, 2, NQ], SCORES_DT, kind="ExternalOutput")
        di["dbg_pos"] = nc.dram_tensor("dbg_pos", [128, 2, 3, W], F32, kind="ExternalOutput")
        di["dbg_mod"] = nc.dram_tensor("dbg_mod", [128, 2], F32, kind="ExternalOutput")
        di["dbg_mlp"] = nc.dram_tensor("dbg_mlp", [128, 2, 3], F32, kind="ExternalOutput")

    with tile.TileContext(nc) as tc:
        _build_tile(nc, tc, di, y)
    nc.compile()
    return nc


MAGIC = 0x5f3759df
I32 = mybir.dt.int32


def _rsqrt_dve(nc, pool, out_ap, x_ap, shape, tag):
    """out = 1/sqrt(x) via bit-trick seed + 2 Newton steps (DVE only).

    x_ap must be an SBUF fp32 AP (read-only); out_ap may alias a target slice.
    """
    y = pool.tile(shape, F32, tag=f"{tag}_y")
    t = pool.tile(shape, F32, tag=f"{tag}_t")
    nc.vector.tensor_scalar(y.bitcast(I32)[:], x_ap.bitcast(I32), 1, None,
                            OP.logical_shift_right)
    nc.vector.tensor_scalar(y.bitcast(I32)[:], y.bitcast(I32)[:], -1, MAGIC,
                            OP.mult, OP.add)
    for _ in range(2):
        nc.vector.tensor_mul(t[:], y[:], y[:])
        nc.vector.tensor_tensor(t[:], t[:], x_ap, OP.mult)
        nc.vector.tensor_scalar(t[:], t[:], -0.5, 1.5, OP.mult, OP.add)
        nc.vector.tensor_tensor(y[:], y[:], t[:], OP.mult)
    nc.vector.tensor_copy(out_ap, y[:])


def _build_tile(nc, tc, di, y):
    with tc.tile_pool(name="cons", bufs=1) as cons:
        # ---- persistent small tiles ----
        ones_sb = cons.tile([128, 1], F32)
        nc.vector.memset(ones_sb[:], 1.0)
        eps1 = cons.tile([1, 1], F32)
        nc.vector.memset(eps1[:], EPS)
        epsc = cons.tile([128, 1], F32)
        nc.vector.memset(epsc[:], EPS)
        epsm = cons.tile([MC, 1], F32)
        nc.vector.memset(epsm[:], EPS)
        ones_r = cons.tile([128, 1], PROJ_DT)
        nc.vector.tensor_copy(ones_r[:], ones_sb[:])
        cv = cons.tile([128, 2, 8], F32)
        pe_sb = cons.tile([128, 2, W], F32)
        qw_sb = cons.tile([128, 2, C], PROJ_DT)
        kw_sb = cons.tile([128, 2, C], PROJ_DT)
        vw_sb = cons.tile([128, 2, C], PROJ_DT)
        ow_sb = cons.tile([128, 2, C], OPROJ_DT)
        rv_sb = cons.tile([128, 2, C], F32)
        vsn_sb = cons.tile([1, C], PROJ_DT)
        sr_sb = cons.tile([1, 2, 2, 128], PROJ_DT)
        gw_sb = cons.tile([128, 2, 1], F32)
        gv_sb = cons.tile([MC, 2], F32)
        id_sb = cons.tile([MC, MC], F32)
        sel_sb = cons.tile([128, 2, NQ], F32)
        posrow = cons.tile([128, 2, 3, W], F32)   # (cc, rowtype, w)
        dtop = cons.tile([128, 2, W], F32)
        dbot = cons.tile([128, 2, W], F32)
        # big persistent state
        tok = cons.tile([128, 2, S], F32)
        tok_r = cons.tile([128, 2, S], PROJ_DT)
        tokq = cons.tile([128, 2, NQ], F32)
        tokq_r = cons.tile([128, 2, NQ], PROJ_DT)
        kst = cons.tile([128, 2, S], SCORES_DT)
        qst = cons.tile([128, 2, NQ], SCORES_DT)
        v_tok = cons.tile([128, 18, NH, 33], AV_DT)
        murow = cons.tile([1, S], PROJ_DT)          # channel-SUM row (256*mu)
        rssc = cons.tile([128, 18], F32)        # rs(key) columns
        qmurow = cons.tile([1, NQ], PROJ_DT)    # q mean row
        rsqbc = cons.tile([128, NQ], F32)       # SCALE * rs_q broadcast
        gate_sb = cons.tile([MC, 6], F32)
        xqres_sb = cons.tile([MC, 6, C], F32)

        with tc.tile_pool(name="work", bufs=2) as wk, \
             tc.tile_pool(name="worksm", bufs=3) as wks, \
             tc.tile_pool(name="pro", bufs=1) as pro, \
             tc.tile_pool(name="att2", bufs=2) as att2, \
             tc.tile_pool(name="atte", bufs=4) as atte, \
             tc.tile_pool(name="avn", bufs=2) as avnp, \
             tc.tile_pool(name="ps_s", bufs=2, space="PSUM") as ps_s, \
             tc.tile_pool(name="ps_av", bufs=1, space="PSUM") as ps_av, \
             tc.tile_pool(name="ps_m", bufs=2, space="PSUM") as ps_m:

            # ---- kick off the big input DMAs first ----
            # four parallel DMA queues: SP(sync) = MLP-critical, ACT = xk/xq,
            # DVE = conv/proj weights, Pool(SWDGE) = cold loads
            xk_re = di["xk"].rearrange("(c p) s -> p c s", p=128)
            nc.scalar.dma_start(out=tokq,
                                in_=di["xq"].rearrange("(c p) s -> p c s", p=128))
            for m in range(NCH):
                nc.scalar.dma_start(out=tok[:, :, m * MEGA:(m + 1) * MEGA],
                                    in_=xk_re[:, :, m * MEGA:(m + 1) * MEGA])
            # ================= text modulation MLP (c-major) =================
            text_sb = pro.tile([128, 4, 1], F32, tag="text")
            nc.sync.dma_start(out=text_sb,
                              in_=di["text"].rearrange("(k p) o -> p k o", p=128))
            w1_sb = pro.tile([128, 4, C], F32, tag="w1")
            nc.sync.dma_start(out=w1_sb,
                              in_=di["tmw1"].rearrange("(k p) d -> p k d", p=128))
            w2_sb = pro.tile([128, 2, C], F32, tag="w2")
            nc.sync.dma_start(out=w2_sb,
                              in_=di["tmw2"].rearrange("(k p) d -> p k d", p=128))
            nc.sync.dma_start(out=cv, in_=di["cvecs"].rearrange("(c p) v -> p c v", p=128))
            nc.sync.dma_start(out=pe_sb, in_=di["pe"].rearrange("(c p) w -> p c w", p=128))
            w3_sb = pro.tile([128, 3, 6, C], PROJ_DT, tag="w3")
            nc.sync.dma_start(out=w3_sb,
                              in_=di["w3b"].rearrange("t (j p) m -> p t j m", p=128))

            def cmajor_mlp_layer(x_col, w_sb, nkc, bias_col, tag):
                h_col = wks.tile([128, 2, 1], F32, tag=f"{tag}_h")
                for c2c in range(2):
                    h_ps = ps_m.tile([128, 1], F32, tag="ps")
                    for kc in range(nkc):
                        nc.tensor.matmul(
                            h_ps[:, :], w_sb[:, kc, c2c * 128:(c2c + 1) * 128],
                            x_col[:, kc, :], start=(kc == 0), stop=(kc == nkc - 1))
                    nc.scalar.activation(h_col[:, c2c, :], h_ps[:, :], AF.Identity,
                                         bias=bias_col[:, c2c, :])
                return h_col

            def cmajor_ln_stats(h_col, tag):
                sum_ps = ps_m.tile([1, 2], F32, tag="ps")
                hsq = wks.tile([128, 2, 1], F32, tag=f"{tag}_hsq")
                nc.vector.tensor_mul(hsq[:], h_col[:], h_col[:])
                for st, src_col in ((0, h_col), (1, hsq)):
                    for cc in range(2):
                        nc.tensor.matmul(sum_ps[:, st:st + 1], ones_sb[:],
                                         src_col[:, cc, :],
                                         start=(cc == 0), stop=(cc == 1))
                ms = wks.tile([1, 2], F32, tag=f"{tag}_ms")
                nc.vector.tensor_scalar_mul(ms[:], sum_ps[:, :], 1.0 / 256.0)
                var1 = wks.tile([1, 1], F32, tag=f"{tag}_var1")
                nc.vector.tensor_mul(var1[:], ms[:, 0:1], ms[:, 0:1])
                nc.vector.scalar_tensor_tensor(var1[:], ms[:, 1:2], EPS, var1[:],
                                               OP.add, OP.subtract)
                _rsqrt_dve(nc, wks, var1[:], var1[:], [1, 1], f"{tag}_rs")
                mu_b = wks.tile([128, 1], F32, tag=f"{tag}_mub")
                nc.gpsimd.partition_broadcast(mu_b[:], ms[:, 0:1])
                rs_b = wks.tile([128, 1], F32, tag=f"{tag}_rsb")
                nc.gpsimd.partition_broadcast(rs_b[:], var1[:])
                return mu_b, rs_b

            h1 = cmajor_mlp_layer(text_sb, w1_sb, 4, cv[:, :, CV_TMB1:CV_TMB1 + 1], "l1")
            mu_b, rs_b = cmajor_ln_stats(h1, "l1")
            h1n = wks.tile([128, 2, 1], F32, tag="h1n")
            for cc in range(2):
                nc.vector.scalar_tensor_tensor(
                    h1n[:, cc, :], h1[:, cc, :], mu_b[:, 0:1], rs_b[:],
                    OP.subtract, OP.mult)
                nc.scalar.activation(h1n[:, cc, :], h1n[:, cc, :], AF.Relu,
                                     bias=cv[:, cc, CV_L1B:CV_L1B + 1],
                                     scale=cv[:, cc, CV_L1G:CV_L1G + 1])
            h2 = cmajor_mlp_layer(h1n, w2_sb, 2, cv[:, :, CV_TMB2:CV_TMB2 + 1], "l2")
            if "dbg_mlp" in di:
                dmlp = cons.tile([128, 2, 3], F32)
                nc.vector.tensor_copy(dmlp[:, :, 0:1], h1[:])
                nc.vector.tensor_copy(dmlp[:, :, 1:2], h1n[:])
                nc.vector.tensor_copy(dmlp[:, :, 2:3], h2[:])
                nc.sync.dma_start(out=di["dbg_mlp"][:, :, :], in_=dmlp[:])
            mu2_b, rs2_b = cmajor_ln_stats(h2, "l2")
            mod = wks.tile([128, 2, 1], F32, tag="mod")
            state_mod = mod
            for cc in range(2):
                nc.vector.scalar_tensor_tensor(
                    mod[:, cc, :], h2[:, cc, :], mu2_b[:, 0:1], rs2_b[:],
                    OP.subtract, OP.mult)
                # sigmoid(z) = 1/(1+exp(-z)) via pre-negated g, b
                nc.scalar.activation(mod[:, cc, :], mod[:, cc, :], AF.Exp,
                                     bias=cv[:, cc, CV_L2BN:CV_L2BN + 1],
                                     scale=cv[:, cc, CV_L2GN:CV_L2GN + 1])
                nc.vector.tensor_scalar(mod[:, cc, :], mod[:, cc, :], 1.0, None, OP.add)
                nc.vector.reciprocal(mod[:, cc, :], mod[:, cc, :])

            # ---- conditional positional rows: 3 distinct conv rows (bf16) ----
            nc.sync.dma_start(out=qw_sb, in_=di["qwg"].rearrange("(c p) d -> p c d", p=128))
            nc.sync.dma_start(out=gw_sb, in_=di["gwg"].rearrange("(c p) o -> p c o", p=128))
            nc.sync.dma_start(out=sel_sb, in_=di["selmask"][:, :, :])
            nc.sync.dma_start(out=kw_sb, in_=di["kwg"].rearrange("(c p) d -> p c d", p=128))
            nc.sync.dma_start(out=vw_sb, in_=di["vwg"].rearrange("(c p) d -> p c d", p=128))
            nc.gpsimd.dma_start(out=sr_sb, in_=di["sumrows"][:, :, :, :])
            nc.gpsimd.dma_start(out=gv_sb, in_=di["gvec"][:, :])
            nc.gpsimd.dma_start(out=id_sb, in_=di["ident"][:, :])
            nc.gpsimd.dma_start(out=vsn_sb, in_=di["vsn"][:, :])
            nc.gpsimd.dma_start(out=rv_sb, in_=di["rowvecs"][:, :, :])
            nc.gpsimd.dma_start(out=ow_sb, in_=di["owg"][:, :, :])
            nc.gpsimd.dma_start(out=xqres_sb,
                                in_=di["xqres"].rearrange("(k p) c -> p k c", p=MC))

            im2 = wks.tile([128, 6, W], PROJ_DT, tag="im2")
            nc.vector.memset(im2[:], 0.0)
            for kw in range(3):
                for cc in range(2):
                    j = kw * 2 + cc
                    if kw == 0:
                        nc.scalar.mul(im2[:, j, 1:W], pe_sb[:, cc, 0:W - 1],
                                      mod[:, cc, 0:1])
                    elif kw == 1:
                        nc.scalar.mul(im2[:, j, :], pe_sb[:, cc, :], mod[:, cc, 0:1])
                    else:
                        nc.scalar.mul(im2[:, j, 0:W - 1], pe_sb[:, cc, 1:W],
                                      mod[:, cc, 0:1])
            cps = ps_m.tile([128, 3, 2, W], F32, tag="ps")
            for t in range(3):
                for oc in range(2):
                    for j in range(6):
                        nc.tensor.matmul(cps[:, t, oc, :],
                                         w3_sb[:, t, j, oc * 128:(oc + 1) * 128],
                                         im2[:, j, :],
                                         start=(j == 0), stop=(j == 5))
            for cc in range(2):
                nc.vector.tensor_scalar(posrow[:, cc, :, :], cps[:, :, cc, :],
                                        cv[:, cc, CV_CONVB:CV_CONVB + 1], None, OP.add)
                nc.vector.tensor_sub(dtop[:, cc, :], posrow[:, cc, 0, :],
                                     posrow[:, cc, 1, :])
                nc.vector.tensor_sub(dbot[:, cc, :], posrow[:, cc, 2, :],
                                     posrow[:, cc, 1, :])

            # ================= q-side: tokens, stats, projections ============
            seltmp = pro.tile([128, NQ], F32, tag="seltmp")
            seltmp2 = pro.tile([128, NQ], F32, tag="seltmp2")
            for cc in range(2):
                eng = nc.vector
                st = seltmp if cc == 0 else seltmp2
                mid = posrow[:, cc, 1:2, :].to_broadcast([128, NQ // W, W])
                tq2 = tokq[:, cc, :].rearrange("p (h w) -> p h w", w=W)
                eng.tensor_tensor(tq2, tq2, mid, OP.add)
                eng.tensor_tensor(
                    st[:].rearrange("p (h w) -> p h w", w=W),
                    sel_sb[:, 0, :].rearrange("p (h w) -> p h w", w=W),
                    dtop[:, cc, None, :].to_broadcast([128, NQ // W, W]), OP.mult)
                eng.tensor_add(tokq[:, cc, :], tokq[:, cc, :], st[:])
                eng.tensor_tensor(
                    st[:].rearrange("p (h w) -> p h w", w=W),
                    sel_sb[:, 1, :].rearrange("p (h w) -> p h w", w=W),
                    dbot[:, cc, None, :].to_broadcast([128, NQ // W, W]), OP.mult)
                eng.tensor_add(tokq[:, cc, :], tokq[:, cc, :], st[:])

            nc.scalar.dma_start(out=tokq_r, in_=tokq.bitcast(PROJ_DT)[:])

            # q stats in column form ([96, 6] chunks)
            sqq = pro.tile([128, 2, NQ], F32, tag="sqq")
            nc.vector.tensor_mul(sqq[:], tokq[:], tokq[:])
            scolq = ps_m.tile([MC, 6, 2], F32, tag="ps")
            for ch in range(6):
                for st, srct in ((0, tokq), (1, sqq)):
                    for cc in range(2):
                        nc.tensor.matmul(scolq[:, ch, st:st + 1],
                                         srct[:, cc, ch * MC:(ch + 1) * MC],
                                         ones_sb[:], start=(cc == 0), stop=(cc == 1))
            mrq = wks.tile([MC, 6, 2], F32, tag="mrq")   # [:,:,0]=mu  [:,:,1]=rs
            nc.vector.tensor_scalar_mul(mrq[:], scolq[:], 1.0 / 256.0)
            varq = wks.tile([MC, 6], F32, tag="varq")
            nc.vector.tensor_mul(varq[:], mrq[:, :, 0], mrq[:, :, 0])
            nc.vector.scalar_tensor_tensor(varq[:], mrq[:, :, 1], EPS, varq[:],
                                           OP.add, OP.subtract)
            _rsqrt_dve(nc, wks, mrq[:, :, 1], varq[:], [MC, 6], "rsq")
            # q mu rows (moving operand for the q-proj correction)
            for half in range(2):
                h0 = half * IT
                mrow_ps = ps_m.tile([1, IT], F32, tag="ps")
                for cc in range(2):
                    nc.tensor.matmul(mrow_ps[:, :], ones_r[:],
                                     tokq_r[:, cc, h0:h0 + IT],
                                     start=(cc == 0), stop=(cc == 1))
                nc.vector.tensor_copy(qmurow[:, h0:h0 + IT], mrow_ps[:, :])
            # rs_q columns -> row via PE transpose, then broadcast
            tp_ps = ps_m.tile([6, MC], F32, tag="ps")
            nc.tensor.transpose(tp_ps[:, :], mrq[:, :, 1], id_sb[:, :])
            tp_sb = pro.tile([6, MC], F32, tag="tpsb")
            nc.vector.tensor_scalar_mul(tp_sb[:], tp_ps[:, :], SCALE)
            qsc = pro.tile([1, NQ], F32, tag="qsc")
            nc.sync.dma_start(
                out=qsc[0:1, :].rearrange("o (j p) -> o j p", p=MC), in_=tp_sb[:, :])
            nc.gpsimd.partition_broadcast(rsqbc[:], qsc[:])

            # q projection (fused LN): psum = qwg^T tokq - mu_q x qwg_sum
            for dc in range(2):
                for half in range(2):
                    h0 = half * IT
                    qp = ps_m.tile([128, IT], F32, tag="ps")
                    for cc in range(2):
                        nc.tensor.matmul(qp[:, :],
                                         qw_sb[:, cc, dc * 128:(dc + 1) * 128],
                                         tokq_r[:, cc, h0:h0 + IT],
                                         start=(cc == 0), stop=False)
                    nc.tensor.matmul(qp[:, :], sr_sb[0:1, 1, dc, :],
                                     qmurow[0:1, h0:h0 + IT],
                                     start=False, stop=True)
                    # qst = SCALE*rs_q (.) psum + qb_total x 1
                    tq = att2.tile([128, IT], F32, tag="tq")
                    nc.vector.tensor_mul(tq[:], qp[:, :], rsqbc[:, h0:h0 + IT])
                    nc.scalar.activation(qst[:, dc, h0:h0 + IT], tq[:], AF.Identity,
                                         bias=cv[:, dc, CV_QBT:CV_QBT + 1])

            # gate logits (fused LN), column form [96, 6]
            gcol = ps_m.tile([MC, 6], F32, tag="ps")
            for ch in range(6):
                for cc in range(2):
                    nc.tensor.matmul(gcol[:, ch:ch + 1],
                                     tokq[:, cc, ch * MC:(ch + 1) * MC],
                                     gw_sb[:, cc, :], start=(cc == 0), stop=(cc == 1))
            glog = wks.tile([MC, 6], F32, tag="glog")
            # glog = rs_q .* (raw - mu_q * gwg_sum) + gb_total
            nc.vector.tensor_scalar_mul(glog[:], mrq[:, :, 0], gv_sb[:, 0:1])
            nc.vector.tensor_sub(glog[:], gcol[:, :], glog[:])
            nc.vector.tensor_mul(glog[:], glog[:], mrq[:, :, 1])
            nc.vector.tensor_scalar(glog[:], glog[:], gv_sb[:, 1:2], None, OP.add)
            eg = wks.tile([MC, 6], F32, tag="eg")
            nc.scalar.activation(eg[:], glog[:], AF.Exp, scale=-1.0)
            nc.vector.tensor_scalar(gate_sb[:], eg[:], 1.0, None, OP.add)
            nc.vector.reciprocal(gate_sb[:], gate_sb[:])

            # ================= k-side mega-chunk pipeline ====================
            def kside_mega(m):
                t0 = m * MEGA
                nrows = MEGA // W  # 24
                for cc in range(2):
                    r0 = 0
                    if m == 0:   # image top row
                        nc.gpsimd.tensor_add(tok[:, cc, 0:W], tok[:, cc, 0:W],
                                             posrow[:, cc, 0, :])
                        r0 = 1
                    r1 = nrows
                    if m == NCH - 1:  # image bottom row
                        nc.gpsimd.tensor_add(tok[:, cc, t0 + MEGA - W:t0 + MEGA],
                                             tok[:, cc, t0 + MEGA - W:t0 + MEGA],
                                             posrow[:, cc, 2, :])
                        r1 = nrows - 1
                    a, b = t0 + r0 * W, t0 + r1 * W
                    mid = posrow[:, cc, 1:2, :].to_broadcast([128, r1 - r0, W])
                    tv = tok[:, cc, a:b].rearrange("p (h w) -> p h w", w=W)
                    nc.gpsimd.tensor_tensor(tv, tv, mid, OP.add)
                # stats columns
                sq = pro.tile([128, 2, MEGA], F32, tag="sqk")
                nc.vector.tensor_mul(sq[:], tok[:, :, t0:t0 + MEGA],
                                     tok[:, :, t0:t0 + MEGA])
                nc.scalar.dma_start(out=tok_r[:, :, t0:t0 + MEGA],
                                    in_=tok.bitcast(PROJ_DT)[:, :, t0:t0 + MEGA])
                scol = ps_m.tile([128, NJM, 2], F32, tag="ps")
                for ch in range(NJM):
                    a = t0 + ch * 128
                    for cc in range(2):
                        nc.tensor.matmul(scol[:, ch, 0:1], tok[:, cc, a:a + 128],
                                         ones_sb[:], start=(cc == 0), stop=(cc == 1))
                    for cc in range(2):
                        nc.tensor.matmul(scol[:, ch, 1:2],
                                         sq[:, cc, ch * 128:(ch + 1) * 128],
                                         ones_sb[:], start=(cc == 0), stop=(cc == 1))
                mm = wks.tile([128, NJM, 2], F32, tag="mmk")
                nc.vector.tensor_scalar_mul(mm[:], scol[:], 1.0 / 256.0)
                var = wks.tile([128, NJM], F32, tag="vark")
                nc.vector.tensor_mul(var[:], mm[:, :, 0], mm[:, :, 0])
                nc.vector.scalar_tensor_tensor(var[:], mm[:, :, 1], EPS, var[:],
                                               OP.add, OP.subtract)
                rsk = rssc[:, m * NJM:(m + 1) * NJM]
                _rsqrt_dve(nc, wks, rsk, var[:], [128, NJM], "rsk")
                # channel-SUM rows (mu-correction moving operand)
                for quar in range(4):
                    a = t0 + quar * IT
                    mrow_ps = ps_m.tile([1, IT], F32, tag="ps")
                    for cc in range(2):
                        nc.tensor.matmul(mrow_ps[:, :], ones_r[:],
                                         tok_r[:, cc, a:a + IT],
                                         start=(cc == 0), stop=(cc == 1))
                    nc.vector.tensor_copy(murow[:, a:a + IT], mrow_ps[:, :])
                # k projection (fused LN, no bias; rs rides in the exp scale)
                for dc in range(2):
                    for quar in range(4):
                        a = t0 + quar * IT
                        kp = ps_m.tile([128, IT], F32, tag="ps")
                        for cc in range(2):
                            nc.tensor.matmul(kp[:, :],
                                             kw_sb[:, cc, dc * 128:(dc + 1) * 128],
                                             tok_r[:, cc, a:a + IT],
                                             start=(cc == 0), stop=False)
                        nc.tensor.matmul(kp[:, :], sr_sb[0:1, 0, dc, :],
                                         murow[0:1, a:a + IT],
                                         start=False, stop=True)
                        nc.scalar.copy(kst[:, dc, a:a + IT], kp[:, :])
                # v projection (fused LN; rs applied on the psum->sbuf copy)
                for ch in range(NJM):
                    jc = m * NJM + ch
                    a = t0 + ch * 128
                    vp = ps_m.tile([128, C], F32, tag="ps")
                    for cc in range(2):
                        nc.tensor.matmul(vp[:, :],
                                         tok_r[:, cc, a:a + 128],
                                         vw_sb[:, cc, :], start=(cc == 0), stop=False)
                    nc.tensor.matmul(vp[:, :], murow[0:1, a:a + 128],
                                     vsn_sb[0:1, :], start=False, stop=True)
                    nc.vector.tensor_scalar(
                        v_tok[:, jc, :, 0:32], vp[:, :].rearrange(
                            "p (h d) -> p h d", d=32),
                        rssc[:, jc:jc + 1], None, OP.mult)
                nc.vector.tensor_copy(
                    v_tok[:, m * NJM:(m + 1) * NJM, :, 32:33],
                    ones_sb[:, None, None, :].to_broadcast([128, NJM, NH, 1]))

            kside_mega(0)

            if "dbg_tok" in di:
                kside_mega(1)
                nc.sync.dma_start(out=di["dbg_tok"][:, :, :], in_=tok[:])
                nc.sync.dma_start(out=di["dbg_kst"][:, :, :], in_=kst[:])
                nc.sync.dma_start(out=di["dbg_gate"][:, :], in_=gate_sb[:])
                nc.sync.dma_start(out=di["dbg_qst"][:, :, :], in_=qst[:])
                nc.sync.dma_start(out=di["dbg_pos"][:, :, :, :], in_=posrow[:])
                nc.sync.dma_start(out=di["dbg_mod"][:, :], in_=state_mod[:, :, 0])

            # ================= attention + epilogue ==========================
            state = {"mod": state_mod}

            def attn_block(it, p, m):
                i0 = it * IT
                av_ps = state["av_ps"]
                for ch in range(NJM):
                    jc = m * NJM + ch
                    s_ps = ps_s.tile([128, 2, 512], F32, tag="sps")
                    for hh in range(2):
                        h = 2 * p + hh
                        dc, poff = h // 4, 32 * (h % 4)
                        nc.tensor.matmul(
                            s_ps[:, hh, 0:IT],
                            kst[poff:poff + 32, dc, jc * 128:(jc + 1) * 128],
                            qst[poff:poff + 32, dc, i0:i0 + IT],
                            start=True, stop=True, tile_position=(poff, 0))
                    e_sb = atte.tile([128, 2, IT], AV_DT, tag="esb")
                    nc.scalar.activation(e_sb[:, :, :], s_ps[:, :, 0:IT],
                                         AF.Exp, scale=rssc[:, jc:jc + 1])
                    for hh in range(2):
                        nc.tensor.matmul(
                            av_ps[:, hh, 0:IT], v_tok[:, jc, 2 * p + hh, :],
                            e_sb[:, hh, :],
                            start=(jc == 0), stop=(jc == 17))

            for it in range(2):
                av_n = avnp.tile([128, 2, IT], OPROJ_DT, tag="avn")
                for p in range(4):
                    av_ps = ps_av.tile([33, 2, 512], F32, tag="avps")
                    state["av_ps"] = av_ps
                    attn_block(it, p, 0)
                    if it == 0 and p == 0 and "dbg_tok" not in di:
                        kside_mega(1)
                    attn_block(it, p, 1)
                    # stage av out of PSUM fast, normalize off the critical path
                    av_ps = state["av_ps"]
                    av_raw = att2.tile([33, 2, IT], F32, tag="avraw")
                    nc.vector.tensor_copy(av_raw[:], av_ps[:, :, 0:IT])
                    r_sb = att2.tile([1, 2, IT], F32, tag="rsb")
                    nc.vector.reciprocal(r_sb[:], av_raw[32:33, :, :])
                    r_bc = att2.tile([32, 2, IT], F32, tag="rbc")
                    nc.gpsimd.partition_broadcast(r_bc[:], r_sb[:])
                    for hh in range(2):
                        h = 2 * p + hh
                        g, poff = h // 4, 32 * (h % 4)
                        nc.vector.tensor_tensor(
                            av_n[poff:poff + 32, g, :],
                            av_raw[0:32, hh, :], r_bc[:, hh, :], OP.mult)
                # output projection + epilogue per 96-token chunk
                for mc in range(3):
                    ch = it * 3 + mc
                    o_ps = ps_m.tile([MC, C], F32, tag="ps")
                    for g in range(2):
                        nc.tensor.matmul(o_ps[:, :],
                                         av_n[:, g, mc * MC:(mc + 1) * MC],
                                         ow_sb[:, g, :], start=(g == 0), stop=(g == 1))
                    og = wk.tile([MC, C], F32, tag="og")
                    nc.vector.tensor_add(og[:], o_ps[:, :], rv_sb[0:MC, RV_OB, :])
                    nc.vector.tensor_scalar_mul(og[:], og[:], gate_sb[:, ch:ch + 1])
                    stats = wks.tile([MC, nc.vector.BN_STATS_DIM], F32, tag="bst")
                    nc.vector.bn_stats(stats[:], og[:])
                    mv = wks.tile([MC, 2], F32, tag="bag")
                    nc.vector.bn_aggr(mv[:], stats[:])
                    rs2 = wks.tile([MC, 1], F32, tag="eprs")
                    nc.vector.tensor_scalar(rs2[:], mv[:, 1:2], EPS, None, OP.add)
                    _rsqrt_dve(nc, wks, rs2[:], rs2[:], [MC, 1], "eprsn")
                    rsn = wks.tile([MC, C], F32, tag="rsn")
                    nc.vector.tensor_scalar_mul(rsn[:], rv_sb[0:MC, RV_NOG, :], rs2[:])
                    t2 = wk.tile([MC, C], F32, tag="ept2")
                    nc.vector.scalar_tensor_tensor(
                        t2[:], og[:], mv[:, 0:1], rsn[:], OP.subtract, OP.mult)
                    nc.vector.tensor_add(t2[:], t2[:], xqres_sb[:, ch, :])
                    nc.sync.dma_start(
                        out=y.rearrange("(k p) c -> p k c", p=MC)[:, ch, :], in_=t2[:])


def _host_inputs(x, text_feature, tm_w1, tm_b1, tm_ln1_g, tm_ln1_b, tm_w2, tm_b2,
                 tm_ln2_g, tm_ln2_b, conv_w, conv_b, q_w, q_b, k_w, k_b, v_w, v_b,
                 o_w, o_b, gate_w, nq_g, nq_b, nkv_g, nkv_b, no_g, no_b):
    f = np.float32
    # pe table (depends only on (c, w); faithful to reference)
    div = np.exp(np.arange(C // 2, dtype=f) * (-math.log(10000.0) / (C // 2)))
    wpos = np.arange(W, dtype=f)
    s = np.sin(wpos[None, :] * div[:, None])
    c = np.cos(wpos[None, :] * div[:, None])
    pe = np.stack([s, c], axis=1).reshape(C, W).astype(f)
    # kh-collapsed conv kernels: top(kh 1,2), mid(all), bot(kh 0,1)
    w3 = np.stack([
        conv_w[:, :, 1, :] + conv_w[:, :, 2, :],
        conv_w.sum(axis=2),
        conv_w[:, :, 0, :] + conv_w[:, :, 1, :],
    ]).astype(f)                                  # [3, Cout, Cin, kw]
    w3 = w3.transpose(0, 3, 2, 1).reshape(3, 768, C)  # [(kw, cin), cout]
    w3b = np.ascontiguousarray(w3, dtype=f)

    # LN-fused projection weights
    qwg = np.ascontiguousarray(q_w.T * nq_g[:, None], dtype=f)   # [c, d]
    kwg = np.ascontiguousarray(k_w.T * nkv_g[:, None], dtype=f)
    vwg = np.ascontiguousarray(v_w.T * nkv_g[:, None], dtype=f)  # [c, o]
    qb_total = (q_b + q_w @ nq_b).astype(f)
    # correction rows pair with channel-SUM rows -> fold the 1/256 here
    sumrows = np.stack([-kwg.sum(axis=0) / 256.0, -qwg.sum(axis=0) / 256.0]) \
        .reshape(1, 2, 2, 128).astype(f)
    # v bias (incl LN beta) folds through softmax-normalized attention
    vb_total = (v_b + v_w @ nkv_b).astype(f)
    ob_eff = (o_b + vb_total @ o_w.T).astype(f)
    # gate
    gwg = np.ascontiguousarray((gate_w[0] * nq_g)[:, None], dtype=f)  # [C, 1]
    gvec = np.zeros((MC, 2), f)
    gvec[:, 0] = gwg.sum()
    gvec[:, 1] = gate_w[0] @ nq_b
    ident = np.eye(MC, dtype=f)
    # head-grouped output projection: partition 32*(h%4)+d, group h//4
    owg = np.zeros((128, 2, C), f)
    for h in range(NH):
        owg[32 * (h % 4):32 * (h % 4) + 32, h // 4, :] = o_w[:, 32 * h:32 * h + 32].T
    if OPROJ_DT == BF16:
        import ml_dtypes
        owg = owg.astype(ml_dtypes.bfloat16)
    rowvecs = np.zeros((128, 2, C), f)
    rowvecs[:, RV_OB, :] = ob_eff[None, :]
    rowvecs[:, RV_NOG, :] = no_g[None, :]
    vsn = np.ascontiguousarray((-vwg.sum(axis=0) / 256.0)[None, :], dtype=f)
    cvecs = np.stack([
        tm_b1, tm_ln1_g, tm_ln1_b, tm_b2, -tm_ln2_g, -tm_ln2_b, conv_b, qb_total,
    ], axis=1).astype(f)                          # [256, 8]

    per_core = []
    for core in range(8):
        b, k = core // 4, core % 4
        xb = np.ascontiguousarray(x[b].reshape(C, S), dtype=f)
        xqc = np.ascontiguousarray(xb[:, NQ * k:NQ * (k + 1)])
        sel = np.zeros((128, 2, NQ), f)
        if k == 0:
            sel[:, 0, 0:W] = 1.0
        if k == 3:
            sel[:, 1, NQ - W:NQ] = 1.0
        per_core.append({
            "xk": xb,
            "xq": xqc,
            "xqres": np.ascontiguousarray(xqc.T + no_b[None, :]),
            "text": np.ascontiguousarray(text_feature[b][:, None], dtype=f),
            "tmw1": np.ascontiguousarray(tm_w1.T, dtype=f),
            "tmw2": np.ascontiguousarray(tm_w2.T, dtype=f),
            "cvecs": cvecs, "pe": pe, "w3b": w3b,
            "qwg": qwg, "kwg": kwg, "vwg": vwg, "owg": owg,
            "rowvecs": rowvecs, "sumrows": sumrows, "vsn": vsn,
            "gwg": gwg, "gvec": gvec, "ident": ident, "selmask": sel,
        })
    return per_core


_NC_CACHE = {}


def get_nc():
    if "nc" not in _NC_CACHE:
        _NC_CACHE["nc"] = build_bass()
    return _NC_CACHE["nc"]


def kernel(**inputs):
    inputs = {k: np.asarray(v, dtype=np.float32) for k, v in inputs.items()}
    in_maps = _host_inputs(**inputs)
    nc = get_nc()
    res = run_bass_kernel_spmd(nc, in_maps, core_ids=list(range(8)))
    x = inputs["x"]
    out = np.empty((B, C, H, W), np.float32)
    for b in range(B):
        blocks = [res.results[4 * b + k]["y"] for k in range(4)]  # [NQ, C] each
        tok = np.concatenate(blocks, axis=0)                      # [S, C]
        out[b] = tok.T.reshape(C, H, W)
    return out


# revision 54
# speedup vs baseline: 1.0001x; 1.0001x over previous
"""Trainium2 Bass kernel for nn_EnhancedTextAttentionBlock (v2).

Self-contained: takes FULL inputs (as in reference.setup_inputs()), shards
across 8 NeuronCores internally, returns the FULL [2, 256, 48, 48] output.

Sharding: core c handles batch b = c // 4 and query-token block k = c % 4
(576 of the 2304 spatial tokens). K/V are computed for the full token set on
every core; a single SPMD program serves all 8 cores with no collectives.

Algebraic restructurings (exact, not approximations):
- pe depends only on (c, w): the 3x3 conv collapses to three 1-D convs (bf16).
- LayerNorms are FUSED into the projections: with kn = (tok - mu)*rs*g + b,
  kst = rs ⊙ (kwg^T tok - mu ⊗ kwg_sum) where kwg = kw*g.  The mu-correction
  rides as an extra K=1 matmul accumulation chunk; the per-token rs rides as
  the exp()'s per-partition scale (keys) or is folded into qst (queries).
  kn/qn are never materialized.
- The k-projection bias shifts every score of a query by a constant and
  cancels in softmax -> dropped.  v's LN beta and bias commute through the
  softmax-normalized attention -> folded into the output bias on host.
- Softmax denominators ride as a ones-column of v; av is normalized before
  a head-grouped output projection (4 heads packed into K=128).
- Softmax max-subtraction is skipped: |scores| < ~2 in fp32 exp range.
- Channel-wise token stats are computed as matmul COLUMNS (free-dim 1), so
  the whole stats scalar chain runs on [128, nchunk] tiles (~free).
"""
import math
import numpy as np

import concourse.bass as bass
import concourse.tile as tile
from concourse import bacc, mybir
from concourse.bass_utils import run_bass_kernel_spmd

import os as _os
F32 = mybir.dt.float32
BF16 = mybir.dt.bfloat16
R = mybir.dt.float32r
_PREC = _os.environ.get("KERNEL_PREC", "allr")
PROJ_DT = F32 if _PREC == "f32" else R      # q/k/v projections (moving = tok)
SCORES_DT = F32 if _PREC in ("f32", "sf32") else R
AV_DT = F32 if _PREC in ("f32", "af32") else BF16   # v / exp(scores) storage
OPROJ_DT = BF16 if _PREC == "obf" else F32
AF = mybir.ActivationFunctionType
OP = mybir.AluOpType

B, C, H, W, T = 2, 256, 48, 48, 512
NH, HD = 8, 32
S = H * W              # 2304 tokens
NQ = S // 4            # 576 q tokens per core
SCALE = HD ** -0.5
IT = 288               # q-tile (two per q block)
MC = 96                # epilogue chunk
MEGA = 1152            # k-side pipeline chunk (2 per S)
NCH = S // MEGA        # 2 mega chunks
NJM = MEGA // 128      # 9 key chunks per mega
EPS = 1e-5

# cvecs column indices (c-major [256, 1] vectors packed into one input)
CV_TMB1, CV_L1G, CV_L1B, CV_TMB2, CV_L2GN, CV_L2BN, CV_CONVB, CV_QBT = range(8)
# rowvecs (token-major prebroadcast [128, 256] rows)
RV_OB, RV_NOG = range(2)


def build_bass():
    nc = bacc.Bacc("TRN2", target_bir_lowering=False, debug=False,
                   enable_asserts=True, num_devices=8)
    di = {}

    def inp(name, shape, dt=F32):
        di[name] = nc.dram_tensor(name, shape, dt, kind="ExternalInput")
        return di[name]

    inp("xk", [C, S])
    inp("xq", [C, NQ])
    inp("xqres", [NQ, C])          # xq^T + no_b (host-folded)
    inp("text", [T, 1])
    inp("tmw1", [T, C])
    inp("tmw2", [C, C])
    inp("cvecs", [C, 8])
    inp("pe", [C, W])
    inp("w3b", [3, 768, C], PROJ_DT)
    inp("qwg", [C, C], PROJ_DT)    # [c, d] = q_w[d, c] * nq_g[c]
    inp("kwg", [C, C], PROJ_DT)
    inp("vwg", [C, C], PROJ_DT)    # [c, o] = v_w[o, c] * nkv_g[c]
    inp("owg", [128, 2, C], OPROJ_DT)
    inp("rowvecs", [128, 2, C])
    inp("vsn", [1, C], PROJ_DT)
    inp("sumrows", [1, 2, 2, 128], PROJ_DT)  # [0, r, dc, d]: r0 = -kwg_sum, r1 = -qwg_sum
    inp("gwg", [C, 1])
    inp("gvec", [MC, 2])           # bc cols: gwg_sum, gb_total
    inp("ident", [MC, MC])
    inp("selmask", [128, 2, NQ])
    y = nc.dram_tensor("y", [NQ, C], F32, kind="ExternalOutput")
    if _os.environ.get("KERNEL_DBG") == "1":
        di["dbg_tok"] = nc.dram_tensor("dbg_tok", [128, 2, S], F32, kind="ExternalOutput")
        di["dbg_kst"] = nc.dram_tensor("dbg_kst", [128, 2, S], SCORES_DT, kind="ExternalOutput")
        di["dbg_gate"] = nc.dram_tensor("dbg_gate", [MC, 6], F32, kind="ExternalOutput")
        di["dbg_qst"] = nc.dram_tensor("dbg_qst", [128, 2, NQ], SCORES_DT, kind="ExternalOutput")
        di["dbg_pos"] = nc.dram_tensor("dbg_pos", [128, 2, 3, W], F32, kind="ExternalOutput")
        di["dbg_mod"] = nc.dram_tensor("dbg_mod", [128, 2], F32, kind="ExternalOutput")
        di["dbg_mlp"] = nc.dram_tensor("dbg_mlp", [128, 2, 3], F32, kind="ExternalOutput")

    with tile.TileContext(nc) as tc:
        _build_tile(nc, tc, di, y)
    nc.compile()
    return nc


MAGIC = 0x5f3759df
I32 = mybir.dt.int32


def _rsqrt_dve(nc, pool, out_ap, x_ap, shape, tag):
    """out = 1/sqrt(x) via bit-trick seed + 2 Newton steps (DVE only).

    x_ap must be an SBUF fp32 AP (read-only); out_ap may alias a target slice.
    """
    y = pool.tile(shape, F32, tag=f"{tag}_y")
    t = pool.tile(shape, F32, tag=f"{tag}_t")
    nc.vector.tensor_scalar(y.bitcast(I32)[:], x_ap.bitcast(I32), 1, None,
                            OP.logical_shift_right)
    nc.vector.tensor_scalar(y.bitcast(I32)[:], y.bitcast(I32)[:], -1, MAGIC,
                            OP.mult, OP.add)
    for _ in range(2):
        nc.vector.tensor_mul(t[:], y[:], y[:])
        nc.vector.tensor_tensor(t[:], t[:], x_ap, OP.mult)
        nc.vector.tensor_scalar(t[:], t[:], -0.5, 1.5, OP.mult, OP.add)
        nc.vector.tensor_tensor(y[:], y[:], t[:], OP.mult)
    nc.vector.tensor_copy(out_ap, y[:])


def _build_tile(nc, tc, di, y):
    with tc.tile_pool(name="cons", bufs=1) as cons:
        # ---- persistent small tiles ----
        ones_sb = cons.tile([128, 1], F32)
        nc.vector.memset(ones_sb[:], 1.0)
        eps1 = cons.tile([1, 1], F32)
        nc.vector.memset(eps1[:], EPS)
        epsc = cons.tile([128, 1], F32)
        nc.vector.memset(epsc[:], EPS)
        epsm = cons.tile([MC, 1], F32)
        nc.vector.memset(epsm[:], EPS)
        ones_r = cons.tile([128, 1], PROJ_DT)
        nc.vector.tensor_copy(ones_r[:], ones_sb[:])
        cv = cons.tile([128, 2, 8], F32)
        pe_sb = cons.tile([128, 2, W], F32)
        qw_sb = cons.tile([128, 2, C], PROJ_DT)
        kw_sb = cons.tile([128, 2, C], PROJ_DT)
        vw_sb = cons.tile([128, 2, C], PROJ_DT)
        ow_sb = cons.tile([128, 2, C], OPROJ_DT)
        rv_sb = cons.tile([128, 2, C], F32)
        vsn_sb = cons.tile([1, C], PROJ_DT)
        sr_sb = cons.tile([1, 2, 2, 128], PROJ_DT)
        gw_sb = cons.tile([128, 2, 1], F32)
        gv_sb = cons.tile([MC, 2], F32)
        id_sb = cons.tile([MC, MC], F32)
        sel_sb = cons.tile([128, 2, NQ], F32)
        posrow = cons.tile([128, 2, 3, W], F32)   # (cc, rowtype, w)
        dtop = cons.tile([128, 2, W], F32)
        dbot = cons.tile([128, 2, W], F32)
        # big persistent state
        tok = cons.tile([128, 2, S], F32)
        tok_r = cons.tile([128, 2, S], PROJ_DT)
        tokq = cons.tile([128, 2, NQ], F32)
        tokq_r = cons.tile([128, 2, NQ], PROJ_DT)
        kst = cons.tile([128, 2, S], SCORES_DT)
        qst = cons.tile([128, 2, NQ], SCORES_DT)
        v_tok = cons.tile([128, 18, NH, 33], AV_DT)
        murow = cons.tile([1, S], PROJ_DT)          # channel-SUM row (256*mu)
        rssc = cons.tile([128, 18], F32)        # rs(key) columns
        qmurow = cons.tile([1, NQ], PROJ_DT)    # q mean row
        rsqbc = cons.tile([128, NQ], F32)       # SCALE * rs_q broadcast
        gate_sb = cons.tile([MC, 6], F32)
        xqres_sb = cons.tile([MC, 6, C], F32)

        with tc.tile_pool(name="work", bufs=2) as wk, \
             tc.tile_pool(name="worksm", bufs=3) as wks, \
             tc.tile_pool(name="pro", bufs=1) as pro, \
             tc.tile_pool(name="att2", bufs=2) as att2, \
             tc.tile_pool(name="atte", bufs=3) as atte, \
             tc.tile_pool(name="avn", bufs=2) as avnp, \
             tc.tile_pool(name="ps_s", bufs=2, space="PSUM") as ps_s, \
             tc.tile_pool(name="ps_av", bufs=1, space="PSUM") as ps_av, \
             tc.tile_pool(name="ps_m", bufs=2, space="PSUM") as ps_m:

            # ---- kick off the big input DMAs first ----
            # four parallel DMA queues: SP(sync) = MLP-critical, ACT = xk/xq,
            # DVE = conv/proj weights, Pool(SWDGE) = cold loads
            xk_re = di["xk"].rearrange("(c p) s -> p c s", p=128)
            nc.scalar.dma_start(out=tokq,
                                in_=di["xq"].rearrange("(c p) s -> p c s", p=128))
            for m in range(NCH):
                nc.scalar.dma_start(out=tok[:, :, m * MEGA:(m + 1) * MEGA],
                                    in_=xk_re[:, :, m * MEGA:(m + 1) * MEGA])
            # ================= text modulation MLP (c-major) =================
            text_sb = pro.tile([128, 4, 1], F32, tag="text")
            nc.sync.dma_start(out=text_sb,
                              in_=di["text"].rearrange("(k p) o -> p k o", p=128))
            w1_sb = pro.tile([128, 4, C], F32, tag="w1")
            nc.sync.dma_start(out=w1_sb,
                              in_=di["tmw1"].rearrange("(k p) d -> p k d", p=128))
            w2_sb = pro.tile([128, 2, C], F32, tag="w2")
            nc.sync.dma_start(out=w2_sb,
                              in_=di["tmw2"].rearrange("(k p) d -> p k d", p=128))
            nc.sync.dma_start(out=cv, in_=di["cvecs"].rearrange("(c p) v -> p c v", p=128))
            nc.sync.dma_start(out=pe_sb, in_=di["pe"].rearrange("(c p) w -> p c w", p=128))
            w3_sb = pro.tile([128, 3, 6, C], PROJ_DT, tag="w3")
            nc.sync.dma_start(out=w3_sb,
                              in_=di["w3b"].rearrange("t (j p) m -> p t j m", p=128))

            def cmajor_mlp_layer(x_col, w_sb, nkc, bias_col, tag):
                h_col = wks.tile([128, 2, 1], F32, tag=f"{tag}_h")
                for c2c in range(2):
                    h_ps = ps_m.tile([128, 1], F32, tag="ps")
                    for kc in range(nkc):
                        nc.tensor.matmul(
                            h_ps[:, :], w_sb[:, kc, c2c * 128:(c2c + 1) * 128],
                            x_col[:, kc, :], start=(kc == 0), stop=(kc == nkc - 1))
                    nc.scalar.activation(h_col[:, c2c, :], h_ps[:, :], AF.Identity,
                                         bias=bias_col[:, c2c, :])
                return h_col

            def cmajor_ln_stats(h_col, tag):
                sum_ps = ps_m.tile([1, 2], F32, tag="ps")
                hsq = wks.tile([128, 2, 1], F32, tag=f"{tag}_hsq")
                nc.vector.tensor_mul(hsq[:], h_col[:], h_col[:])
                for st, src_col in ((0, h_col), (1, hsq)):
                    for cc in range(2):
                        nc.tensor.matmul(sum_ps[:, st:st + 1], ones_sb[:],
                                         src_col[:, cc, :],
                                         start=(cc == 0), stop=(cc == 1))
                ms = wks.tile([1, 2], F32, tag=f"{tag}_ms")
                nc.vector.tensor_scalar_mul(ms[:], sum_ps[:, :], 1.0 / 256.0)
                var1 = wks.tile([1, 1], F32, tag=f"{tag}_var1")
                nc.vector.tensor_mul(var1[:], ms[:, 0:1], ms[:, 0:1])
                nc.vector.scalar_tensor_tensor(var1[:], ms[:, 1:2], EPS, var1[:],
                                               OP.add, OP.subtract)
                _rsqrt_dve(nc, wks, var1[:], var1[:], [1, 1], f"{tag}_rs")
                mu_b = wks.tile([128, 1], F32, tag=f"{tag}_mub")
                nc.gpsimd.partition_broadcast(mu_b[:], ms[:, 0:1])
                rs_b = wks.tile([128, 1], F32, tag=f"{tag}_rsb")
                nc.gpsimd.partition_broadcast(rs_b[:], var1[:])
                return mu_b, rs_b

            h1 = cmajor_mlp_layer(text_sb, w1_sb, 4, cv[:, :, CV_TMB1:CV_TMB1 + 1], "l1")
            mu_b, rs_b = cmajor_ln_stats(h1, "l1")
            h1n = wks.tile([128, 2, 1], F32, tag="h1n")
            for cc in range(2):
                nc.vector.scalar_tensor_tensor(
                    h1n[:, cc, :], h1[:, cc, :], mu_b[:, 0:1], rs_b[:],
                    OP.subtract, OP.mult)
                nc.scalar.activation(h1n[:, cc, :], h1n[:, cc, :], AF.Relu,
                                     bias=cv[:, cc, CV_L1B:CV_L1B + 1],
                                     scale=cv[:, cc, CV_L1G:CV_L1G + 1])
            h2 = cmajor_mlp_layer(h1n, w2_sb, 2, cv[:, :, CV_TMB2:CV_TMB2 + 1], "l2")
            if "dbg_mlp" in di:
                dmlp = cons.tile([128, 2, 3], F32)
                nc.vector.tensor_copy(dmlp[:, :, 0:1], h1[:])
                nc.vector.tensor_copy(dmlp[:, :, 1:2], h1n[:])
                nc.vector.tensor_copy(dmlp[:, :, 2:3], h2[:])
                nc.sync.dma_start(out=di["dbg_mlp"][:, :, :], in_=dmlp[:])
            mu2_b, rs2_b = cmajor_ln_stats(h2, "l2")
            mod = wks.tile([128, 2, 1], F32, tag="mod")
            state_mod = mod
            for cc in range(2):
                nc.vector.scalar_tensor_tensor(
                    mod[:, cc, :], h2[:, cc, :], mu2_b[:, 0:1], rs2_b[:],
                    OP.subtract, OP.mult)
                # sigmoid(z) = 1/(1+exp(-z)) via pre-negated g, b
                nc.scalar.activation(mod[:, cc, :], mod[:, cc, :], AF.Exp,
                                     bias=cv[:, cc, CV_L2BN:CV_L2BN + 1],
                                     scale=cv[:, cc, CV_L2GN:CV_L2GN + 1])
                nc.vector.tensor_scalar(mod[:, cc, :], mod[:, cc, :], 1.0, None, OP.add)
                nc.vector.reciprocal(mod[:, cc, :], mod[:, cc, :])

            # ---- conditional positional rows: 3 distinct conv rows (bf16) ----
            nc.sync.dma_start(out=qw_sb, in_=di["qwg"].rearrange("(c p) d -> p c d", p=128))
            nc.sync.dma_start(out=gw_sb, in_=di["gwg"].rearrange("(c p) o -> p c o", p=128))
            nc.sync.dma_start(out=sel_sb, in_=di["selmask"][:, :, :])
            nc.sync.dma_start(out=kw_sb, in_=di["kwg"].rearrange("(c p) d -> p c d", p=128))
            nc.sync.dma_start(out=vw_sb, in_=di["vwg"].rearrange("(c p) d -> p c d", p=128))
            nc.gpsimd.dma_start(out=sr_sb, in_=di["sumrows"][:, :, :, :])
            nc.gpsimd.dma_start(out=gv_sb, in_=di["gvec"][:, :])
            nc.gpsimd.dma_start(out=id_sb, in_=di["ident"][:, :])
            nc.gpsimd.dma_start(out=vsn_sb, in_=di["vsn"][:, :])
            nc.gpsimd.dma_start(out=rv_sb, in_=di["rowvecs"][:, :, :])
            nc.gpsimd.dma_start(out=ow_sb, in_=di["owg"][:, :, :])
            nc.gpsimd.dma_start(out=xqres_sb,
                                in_=di["xqres"].rearrange("(k p) c -> p k c", p=MC))

            im2 = wks.tile([128, 6, W], PROJ_DT, tag="im2")
            nc.vector.memset(im2[:], 0.0)
            for kw in range(3):
                for cc in range(2):
                    j = kw * 2 + cc
                    if kw == 0:
                        nc.scalar.mul(im2[:, j, 1:W], pe_sb[:, cc, 0:W - 1],
                                      mod[:, cc, 0:1])
                    elif kw == 1:
                        nc.scalar.mul(im2[:, j, :], pe_sb[:, cc, :], mod[:, cc, 0:1])
                    else:
                        nc.scalar.mul(im2[:, j, 0:W - 1], pe_sb[:, cc, 1:W],
                                      mod[:, cc, 0:1])
            cps = ps_m.tile([128, 3, 2, W], F32, tag="ps")
            for t in range(3):
                for oc in range(2):
                    for j in range(6):
                        nc.tensor.matmul(cps[:, t, oc, :],
                                         w3_sb[:, t, j, oc * 128:(oc + 1) * 128],
                                         im2[:, j, :],
                                         start=(j == 0), stop=(j == 5))
            for cc in range(2):
                nc.vector.tensor_scalar(posrow[:, cc, :, :], cps[:, :, cc, :],
                                        cv[:, cc, CV_CONVB:CV_CONVB + 1], None, OP.add)
                nc.vector.tensor_sub(dtop[:, cc, :], posrow[:, cc, 0, :],
                                     posrow[:, cc, 1, :])
                nc.vector.tensor_sub(dbot[:, cc, :], posrow[:, cc, 2, :],
                                     posrow[:, cc, 1, :])

            # ================= q-side: tokens, stats, projections ============
            seltmp = pro.tile([128, NQ], F32, tag="seltmp")
            seltmp2 = pro.tile([128, NQ], F32, tag="seltmp2")
            for cc in range(2):
                eng = nc.vector
                st = seltmp if cc == 0 else seltmp2
                mid = posrow[:, cc, 1:2, :].to_broadcast([128, NQ // W, W])
                tq2 = tokq[:, cc, :].rearrange("p (h w) -> p h w", w=W)
                eng.tensor_tensor(tq2, tq2, mid, OP.add)
                eng.tensor_tensor(
                    st[:].rearrange("p (h w) -> p h w", w=W),
                    sel_sb[:, 0, :].rearrange("p (h w) -> p h w", w=W),
                    dtop[:, cc, None, :].to_broadcast([128, NQ // W, W]), OP.mult)
                eng.tensor_add(tokq[:, cc, :], tokq[:, cc, :], st[:])
                eng.tensor_tensor(
                    st[:].rearrange("p (h w) -> p h w", w=W),
                    sel_sb[:, 1, :].rearrange("p (h w) -> p h w", w=W),
                    dbot[:, cc, None, :].to_broadcast([128, NQ // W, W]), OP.mult)
                eng.tensor_add(tokq[:, cc, :], tokq[:, cc, :], st[:])

            nc.scalar.dma_start(out=tokq_r, in_=tokq.bitcast(PROJ_DT)[:])

            # q stats in column form ([96, 6] chunks)
            sqq = pro.tile([128, 2, NQ], F32, tag="sqq")
            nc.vector.tensor_mul(sqq[:], tokq[:], tokq[:])
            scolq = ps_m.tile([MC, 6, 2], F32, tag="ps")
            for ch in range(6):
                for st, srct in ((0, tokq), (1, sqq)):
                    for cc in range(2):
                        nc.tensor.matmul(scolq[:, ch, st:st + 1],
                                         srct[:, cc, ch * MC:(ch + 1) * MC],
                                         ones_sb[:], start=(cc == 0), stop=(cc == 1))
            mrq = wks.tile([MC, 6, 2], F32, tag="mrq")   # [:,:,0]=mu  [:,:,1]=rs
            nc.vector.tensor_scalar_mul(mrq[:], scolq[:], 1.0 / 256.0)
            varq = wks.tile([MC, 6], F32, tag="varq")
            nc.vector.tensor_mul(varq[:], mrq[:, :, 0], mrq[:, :, 0])
            nc.vector.scalar_tensor_tensor(varq[:], mrq[:, :, 1], EPS, varq[:],
                                           OP.add, OP.subtract)
            _rsqrt_dve(nc, wks, mrq[:, :, 1], varq[:], [MC, 6], "rsq")
            # q mu rows (moving operand for the q-proj correction)
            for half in range(2):
                h0 = half * IT
                mrow_ps = ps_m.tile([1, IT], F32, tag="ps")
                for cc in range(2):
                    nc.tensor.matmul(mrow_ps[:, :], ones_r[:],
                                     tokq_r[:, cc, h0:h0 + IT],
                                     start=(cc == 0), stop=(cc == 1))
                nc.vector.tensor_copy(qmurow[:, h0:h0 + IT], mrow_ps[:, :])
            # rs_q columns -> row via PE transpose, then broadcast
            tp_ps = ps_m.tile([6, MC], F32, tag="ps")
            nc.tensor.transpose(tp_ps[:, :], mrq[:, :, 1], id_sb[:, :])
            tp_sb = pro.tile([6, MC], F32, tag="tpsb")
            nc.vector.tensor_scalar_mul(tp_sb[:], tp_ps[:, :], SCALE)
            qsc = pro.tile([1, NQ], F32, tag="qsc")
            nc.sync.dma_start(
                out=qsc[0:1, :].rearrange("o (j p) -> o j p", p=MC), in_=tp_sb[:, :])
            nc.gpsimd.partition_broadcast(rsqbc[:], qsc[:])

            # q projection (fused LN): psum = qwg^T tokq - mu_q x qwg_sum
            for dc in range(2):
                for half in range(2):
                    h0 = half * IT
                    qp = ps_m.tile([128, IT], F32, tag="ps")
                    for cc in range(2):
                        nc.tensor.matmul(qp[:, :],
                                         qw_sb[:, cc, dc * 128:(dc + 1) * 128],
                                         tokq_r[:, cc, h0:h0 + IT],
                                         start=(cc == 0), stop=False)
                    nc.tensor.matmul(qp[:, :], sr_sb[0:1, 1, dc, :],
                                     qmurow[0:1, h0:h0 + IT],
                                     start=False, stop=True)
                    # qst = SCALE*rs_q (.) psum + qb_total x 1
                    tq = att2.tile([128, IT], F32, tag="tq")
                    nc.vector.tensor_mul(tq[:], qp[:, :], rsqbc[:, h0:h0 + IT])
                    nc.scalar.activation(qst[:, dc, h0:h0 + IT], tq[:], AF.Identity,
                                         bias=cv[:, dc, CV_QBT:CV_QBT + 1])

            # gate logits (fused LN), column form [96, 6]
            gcol = ps_m.tile([MC, 6], F32, tag="ps")
            for ch in range(6):
                for cc in range(2):
                    nc.tensor.matmul(gcol[:, ch:ch + 1],
                                     tokq[:, cc, ch * MC:(ch + 1) * MC],
                                     gw_sb[:, cc, :], start=(cc == 0), stop=(cc == 1))
            glog = wks.tile([MC, 6], F32, tag="glog")
            # glog = rs_q .* (raw - mu_q * gwg_sum) + gb_total
            nc.vector.tensor_scalar_mul(glog[:], mrq[:, :, 0], gv_sb[:, 0:1])
            nc.vector.tensor_sub(glog[:], gcol[:, :], glog[:])
            nc.vector.tensor_mul(glog[:], glog[:], mrq[:, :, 1])
            nc.vector.tensor_scalar(glog[:], glog[:], gv_sb[:, 1:2], None, OP.add)
            eg = wks.tile([MC, 6], F32, tag="eg")
            nc.scalar.activation(eg[:], glog[:], AF.Exp, scale=-1.0)
            nc.vector.tensor_scalar(gate_sb[:], eg[:], 1.0, None, OP.add)
            nc.vector.reciprocal(gate_sb[:], gate_sb[:])

            # ================= k-side mega-chunk pipeline ====================
            def kside_mega(m):
                t0 = m * MEGA
                nrows = MEGA // W  # 24
                for cc in range(2):
                    r0 = 0
                    if m == 0:   # image top row
                        nc.gpsimd.tensor_add(tok[:, cc, 0:W], tok[:, cc, 0:W],
                                             posrow[:, cc, 0, :])
                        r0 = 1
                    r1 = nrows
                    if m == NCH - 1:  # image bottom row
                        nc.gpsimd.tensor_add(tok[:, cc, t0 + MEGA - W:t0 + MEGA],
                                             tok[:, cc, t0 + MEGA - W:t0 + MEGA],
                                             posrow[:, cc, 2, :])
                        r1 = nrows - 1
                    a, b = t0 + r0 * W, t0 + r1 * W
                    mid = posrow[:, cc, 1:2, :].to_broadcast([128, r1 - r0, W])
                    tv = tok[:, cc, a:b].rearrange("p (h w) -> p h w", w=W)
                    nc.gpsimd.tensor_tensor(tv, tv, mid, OP.add)
                # stats columns
                sq = pro.tile([128, 2, MEGA], F32, tag="sqk")
                nc.vector.tensor_mul(sq[:], tok[:, :, t0:t0 + MEGA],
                                     tok[:, :, t0:t0 + MEGA])
                nc.scalar.dma_start(out=tok_r[:, :, t0:t0 + MEGA],
                                    in_=tok.bitcast(PROJ_DT)[:, :, t0:t0 + MEGA])
                scol = ps_m.tile([128, NJM, 2], F32, tag="ps")
                for ch in range(NJM):
                    a = t0 + ch * 128
                    for cc in range(2):
                        nc.tensor.matmul(scol[:, ch, 0:1], tok[:, cc, a:a + 128],
                                         ones_sb[:], start=(cc == 0), stop=(cc == 1))
                    for cc in range(2):
                        nc.tensor.matmul(scol[:, ch, 1:2],
                                         sq[:, cc, ch * 128:(ch + 1) * 128],
                                         ones_sb[:], start=(cc == 0), stop=(cc == 1))
                mm = wks.tile([128, NJM, 2], F32, tag="mmk")
                nc.vector.tensor_scalar_mul(mm[:], scol[:], 1.0 / 256.0)
                var = wks.tile([128, NJM], F32, tag="vark")
                nc.vector.tensor_mul(var[:], mm[:, :, 0], mm[:, :, 0])
                nc.vector.scalar_tensor_tensor(var[:], mm[:, :, 1], EPS, var[:],
                                               OP.add, OP.subtract)
                rsk = rssc[:, m * NJM:(m + 1) * NJM]
                _rsqrt_dve(nc, wks, rsk, var[:], [128, NJM], "rsk")
                # channel-SUM rows (mu-correction moving operand)
                for quar in range(4):
                    a = t0 + quar * IT
                    mrow_ps = ps_m.tile([1, IT], F32, tag="ps")
                    for cc in range(2):
                        nc.tensor.matmul(mrow_ps[:, :], ones_r[:],
                                         tok_r[:, cc, a:a + IT],
                                         start=(cc == 0), stop=(cc == 1))
                    nc.vector.tensor_copy(murow[:, a:a + IT], mrow_ps[:, :])
                # k projection (fused LN, no bias; rs rides in the exp scale)
                for dc in range(2):
                    for quar in range(4):
                        a = t0 + quar * IT
                        kp = ps_m.tile([128, IT], F32, tag="ps")
                        for cc in range(2):
                            nc.tensor.matmul(kp[:, :],
                                             kw_sb[:, cc, dc * 128:(dc + 1) * 128],
                                             tok_r[:, cc, a:a + IT],
                                             start=(cc == 0), stop=False)
                        nc.tensor.matmul(kp[:, :], sr_sb[0:1, 0, dc, :],
                                         murow[0:1, a:a + IT],
                                         start=False, stop=True)
                        nc.scalar.copy(kst[:, dc, a:a + IT], kp[:, :])
                # v projection (fused LN; rs applied on the psum->sbuf copy)
                for ch in range(NJM):
                    jc = m * NJM + ch
                    a = t0 + ch * 128
                    vp = ps_m.tile([128, C], F32, tag="ps")
                    for cc in range(2):
                        nc.tensor.matmul(vp[:, :],
                                         tok_r[:, cc, a:a + 128],
                                         vw_sb[:, cc, :], start=(cc == 0), stop=False)
                    nc.tensor.matmul(vp[:, :], murow[0:1, a:a + 128],
                                     vsn_sb[0:1, :], start=False, stop=True)
                    nc.vector.tensor_scalar(
                        v_tok[:, jc, :, 0:32], vp[:, :].rearrange(
                            "p (h d) -> p h d", d=32),
                        rssc[:, jc:jc + 1], None, OP.mult)
                nc.vector.tensor_copy(
                    v_tok[:, m * NJM:(m + 1) * NJM, :, 32:33],
                    ones_sb[:, None, None, :].to_broadcast([128, NJM, NH, 1]))

            kside_mega(0)

            if "dbg_tok" in di:
                kside_mega(1)
                nc.sync.dma_start(out=di["dbg_tok"][:, :, :], in_=tok[:])
                nc.sync.dma_start(out=di["dbg_kst"][:, :, :], in_=kst[:])
                nc.sync.dma_start(out=di["dbg_gate"][:, :], in_=gate_sb[:])
                nc.sync.dma_start(out=di["dbg_qst"][:, :, :], in_=qst[:])
                nc.sync.dma_start(out=di["dbg_pos"][:, :, :, :], in_=posrow[:])
                nc.sync.dma_start(out=di["dbg_mod"][:, :], in_=state_mod[:, :, 0])

            # ================= attention + epilogue ==========================
            state = {"mod": state_mod}

            def attn_block(it, p, m):
                i0 = it * IT
                av_ps = state["av_ps"]
                for ch in range(NJM):
                    jc = m * NJM + ch
                    s_ps = ps_s.tile([128, 2, 512], F32, tag="sps")
                    for hh in range(2):
                        h = 2 * p + hh
                        dc, poff = h // 4, 32 * (h % 4)
                        nc.tensor.matmul(
                            s_ps[:, hh, 0:IT],
                            kst[poff:poff + 32, dc, jc * 128:(jc + 1) * 128],
                            qst[poff:poff + 32, dc, i0:i0 + IT],
                            start=True, stop=True, tile_position=(poff, 0))
                    e_sb = atte.tile([128, 2, IT], AV_DT, tag="esb")
                    nc.scalar.activation(e_sb[:, :, :], s_ps[:, :, 0:IT],
                                         AF.Exp, scale=rssc[:, jc:jc + 1])
                    for hh in range(2):
                        nc.tensor.matmul(
                            av_ps[:, hh, 0:IT], v_tok[:, jc, 2 * p + hh, :],
                            e_sb[:, hh, :],
                            start=(jc == 0), stop=(jc == 17))

            for it in range(2):
                av_n = avnp.tile([128, 2, IT], OPROJ_DT, tag="avn")
                for p in range(4):
                    av_ps = ps_av.tile([33, 2, 512], F32, tag="avps")
                    state["av_ps"] = av_ps
                    attn_block(it, p, 0)
                    if it == 0 and p == 0 and "dbg_tok" not in di:
                        kside_mega(1)
                    attn_block(it, p, 1)
                    # stage av out of PSUM fast, normalize off the critical path
                    av_ps = state["av_ps"]
                    av_raw = att2.tile([33, 2, IT], F32, tag="avraw")
                    nc.vector.tensor_copy(av_raw[:], av_ps[:, :, 0:IT])
                    r_sb = att2.tile([1, 2, IT], F32, tag="rsb")
                    nc.vector.reciprocal(r_sb[:], av_raw[32:33, :, :])
                    r_bc = att2.tile([32, 2, IT], F32, tag="rbc")
                    nc.gpsimd.partition_broadcast(r_bc[:], r_sb[:])
                    for hh in range(2):
                        h = 2 * p + hh
                        g, poff = h // 4, 32 * (h % 4)
                        nc.vector.tensor_tensor(
                            av_n[poff:poff + 32, g, :],
                            av_raw[0:32, hh, :], r_bc[:, hh, :], OP.mult)
                # output projection + epilogue per 96-token chunk
                for mc in range(3):
                    ch = it * 3 + mc
                    o_ps = ps_m.tile([MC, C], F32, tag="ps")
                    for g in range(2):
                        nc.tensor.matmul(o_ps[:, :],
                                         av_n[:, g, mc * MC:(mc + 1) * MC],
                                         ow_sb[:, g, :], start=(g == 0), stop=(g == 1))
                    og = wk.tile([MC, C], F32, tag="og")
                    nc.vector.tensor_add(og[:], o_ps[:, :], rv_sb[0:MC, RV_OB, :])
                    nc.vector.tensor_scalar_mul(og[:], og[:], gate_sb[:, ch:ch + 1])
                    stats = wks.tile([MC, nc.vector.BN_STATS_DIM], F32, tag="bst")
                    nc.vector.bn_stats(stats[:], og[:])
                    mv = wks.tile([MC, 2], F32, tag="bag")
                    nc.vector.bn_aggr(mv[:], stats[:])
                    rs2 = wks.tile([MC, 1], F32, tag="eprs")
                    nc.vector.tensor_scalar(rs2[:], mv[:, 1:2], EPS, None, OP.add)
                    _rsqrt_dve(nc, wks, rs2[:], rs2[:], [MC, 1], "eprsn")
                    rsn = wks.tile([MC, C], F32, tag="rsn")
                    nc.vector.tensor_scalar_mul(rsn[:], rv_sb[0:MC, RV_NOG, :], rs2[:])
                    t2 = wk.tile([MC, C], F32, tag="ept2")
                    nc.vector.scalar_tensor_tensor(
                        t2[:], og[:], mv[:, 0:1], rsn[:], OP.subtract, OP.mult)
                    nc.vector.tensor_add(t2[:], t2[:], xqres_sb[:, ch, :])
                    nc.sync.dma_start(
                        out=y.rearrange("(k p) c -> p k c", p=MC)[:, ch, :], in_=t2[:])


def _host_inputs(x, text_feature, tm_w1, tm_b1, tm_ln1_g, tm_ln1_b, tm_w2, tm_b2,
                 tm_ln2_g, tm_ln2_b, conv_w, conv_b, q_w, q_b, k_w, k_b, v_w, v_b,
                 o_w, o_b, gate_w, nq_g, nq_b, nkv_g, nkv_b, no_g, no_b):
    f = np.float32
    # pe table (depends only on (c, w); faithful to reference)
    div = np.exp(np.arange(C // 2, dtype=f) * (-math.log(10000.0) / (C // 2)))
    wpos = np.arange(W, dtype=f)
    s = np.sin(wpos[None, :] * div[:, None])
    c = np.cos(wpos[None, :] * div[:, None])
    pe = np.stack([s, c], axis=1).reshape(C, W).astype(f)
    # kh-collapsed conv kernels: top(kh 1,2), mid(all), bot(kh 0,1)
    w3 = np.stack([
        conv_w[:, :, 1, :] + conv_w[:, :, 2, :],
        conv_w.sum(axis=2),
        conv_w[:, :, 0, :] + conv_w[:, :, 1, :],
    ]).astype(f)                                  # [3, Cout, Cin, kw]
    w3 = w3.transpose(0, 3, 2, 1).reshape(3, 768, C)  # [(kw, cin), cout]
    w3b = np.ascontiguousarray(w3, dtype=f)

    # LN-fused projection weights
    qwg = np.ascontiguousarray(q_w.T * nq_g[:, None], dtype=f)   # [c, d]
    kwg = np.ascontiguousarray(k_w.T * nkv_g[:, None], dtype=f)
    vwg = np.ascontiguousarray(v_w.T * nkv_g[:, None], dtype=f)  # [c, o]
    qb_total = (q_b + q_w @ nq_b).astype(f)
    # correction rows pair with channel-SUM rows -> fold the 1/256 here
    sumrows = np.stack([-kwg.sum(axis=0) / 256.0, -qwg.sum(axis=0) / 256.0]) \
        .reshape(1, 2, 2, 128).astype(f)
    # v bias (incl LN beta) folds through softmax-normalized attention
    vb_total = (v_b + v_w @ nkv_b).astype(f)
    ob_eff = (o_b + vb_total @ o_w.T).astype(f)
    # gate
    gwg = np.ascontiguousarray((gate_w[0] * nq_g)[:, None], dtype=f)  # [C, 1]
    gvec = np.zeros((MC, 2), f)
    gvec[:, 0] = gwg.sum()
    gvec[:, 1] = gate_w[0] @ nq_b
    ident = np.eye(MC, dtype=f)
    # head-grouped output projection: partition 32*(h%4)+d, group h//4
    owg = np.zeros((128, 2, C), f)
    for h in range(NH):
        owg[32 * (h % 4):32 * (h % 4) + 32, h // 4, :] = o_w[:, 32 * h:32 * h + 32].T
    if OPROJ_DT == BF16:
        import ml_dtypes
        owg = owg.astype(ml_dtypes.bfloat16)
    rowvecs = np.zeros((128, 2, C), f)
    rowvecs[:, RV_OB, :] = ob_eff[None, :]
    rowvecs[:, RV_NOG, :] = no_g[None, :]
    vsn = np.ascontiguousarray((-vwg.sum(axis=0) / 256.0)[None, :], dtype=f)
    cvecs = np.stack([
        tm_b1, tm_ln1_g, tm_ln1_b, tm_b2, -tm_ln2_g, -tm_ln2_b, conv_b, qb_total,
    ], axis=1).astype(f)                          # [256, 8]

    per_core = []
    for core in range(8):
        b, k = core // 4, core % 4
        xb = np.ascontiguousarray(x[b].reshape(C, S), dtype=f)
        xqc = np.ascontiguousarray(xb[:, NQ * k:NQ * (k + 1)])
        sel = np.zeros((128, 2, NQ), f)
        if k == 0:
            sel[:, 0, 0:W] = 1.0
        if k == 3:
            sel[:, 1, NQ - W:NQ] = 1.0
        per_core.append({
            "xk": xb,
            "xq": xqc,
            "xqres": np.ascontiguousarray(xqc.T + no_b[None, :]),
            "text": np.ascontiguousarray(text_feature[b][:, None], dtype=f),
            "tmw1": np.ascontiguousarray(tm_w1.T, dtype=f),
            "tmw2": np.ascontiguousarray(tm_w2.T, dtype=f),
            "cvecs": cvecs, "pe": pe, "w3b": w3b,
            "qwg": qwg, "kwg": kwg, "vwg": vwg, "owg": owg,
            "rowvecs": rowvecs, "sumrows": sumrows, "vsn": vsn,
            "gwg": gwg, "gvec": gvec, "ident": ident, "selmask": sel,
        })
    return per_core


_NC_CACHE = {}


def get_nc():
    if "nc" not in _NC_CACHE:
        _NC_CACHE["nc"] = build_bass()
    return _NC_CACHE["nc"]


def kernel(**inputs):
    inputs = {k: np.asarray(v, dtype=np.float32) for k, v in inputs.items()}
    in_maps = _host_inputs(**inputs)
    nc = get_nc()
    res = run_bass_kernel_spmd(nc, in_maps, core_ids=list(range(8)))
    x = inputs["x"]
    out = np.empty((B, C, H, W), np.float32)
    for b in range(B):
        blocks = [res.results[4 * b + k]["y"] for k in range(4)]  # [NQ, C] each
        tok = np.concatenate(blocks, axis=0)                      # [S, C]
        out[b] = tok.T.reshape(C, H, W)
    return out
